# revision 25
# baseline (speedup 1.0000x reference)
"""Trainium2 Bass kernel for ClosebyValuationFunction.

reference semantics (per row r of two [B, 6] f32 tensors):
    dis_x = |z1[r,4] - z2[r,4]|; dis_y = |z1[r,5] - z2[r,5]|
    out[r] = 0.99 if (dis_x < 2.0) & (dis_y <= 0.1) else 0.01

Strategy: data-parallel over 8 cores (B/8 rows each). Only columns 4/5
of each input participate, so the shard each core receives is the
projection of its row range onto those columns, packed planar-pairs as
[2, N, 2] (plane 0 = z1's (x,y) pairs, plane 1 = z2's) — the host does
the slice/pack while sharding; every arithmetic op (subtract, abs,
compare, select) runs on device. Per core that is 16 MiB in + 4 MiB
out of HBM traffic instead of the 52 MiB of full rows.

Per chunk the compute is spread so no engine exceeds the DMA time:
one fused DVE subtract over both planes, |.| on ACT in one op, then
two DVE ops — cx = (|dx| < 2)*0.98 (tensor_scalar) and
res0 = (|dy| <= 0.1)*cx (scalar_tensor_tensor) — and the final
res = res0 + 0.01 on ACT (Identity+bias), which also issues the
store. DVE ~4.4us/chunk, ACT ~3.6us/chunk, DMA ~6.4us/chunk.

Input DMAs ride the Sync HWDGE queue; output DMAs ride the ACT HWDGE
queue so a compute-gated store never stalls the input stream (HWDGE is
FIFO per issuing engine). The last chunk is tapered into small
sub-chunks to shrink the kernel tail.
"""

import numpy as np

B = 8388608
M = 8            # cores
N = B // M       # rows per core
P = 128          # partitions
E = 1024         # rows per partition per full chunk
E_TAIL = 256     # rows per partition per tail sub-chunk

HI = 0.99
LO = 0.01
X_THRESH = 2.0
Y_NEXT = float(np.nextafter(np.float32(0.1), np.float32(1)))  # |dy|<=0.1 == |dy|<Y_NEXT

_cache: dict = {}


def _build(n_rows: int = N, e: int = E, e_tail: int = E_TAIL,
           io_bufs: int = 3, tail_bufs: int = 8, tmp_bufs: int = 3):
    from concourse import bacc, mybir
    from concourse.tile import TileContext

    f32 = mybir.dt.float32
    Alu = mybir.AluOpType
    Act = mybir.ActivationFunctionType

    n_chunks = n_rows // (P * e)
    assert n_chunks * P * e == n_rows
    assert e % e_tail == 0

    nc = bacc.Bacc("TRN2", target_bir_lowering=False, debug=False)

    xy = nc.dram_tensor("xy", [2, n_rows, 2], f32, kind="ExternalInput")
    out = nc.dram_tensor("out", [n_rows], f32, kind="ExternalOutput")

    # full chunks: chunk c, partition p holds rows [(c*P + p)*e, ...) of
    # both planes; SBUF free layout = [plane0 pairs (2e)][plane1 pairs (2e)]
    z1c = xy[0].rearrange("(c p e) d -> c p (e d)", p=P, e=e)
    z2c = xy[1].rearrange("(c p e) d -> c p (e d)", p=P, e=e)
    outt = out[:].rearrange("(c p e) -> c p e", p=P, e=e)

    # geometric taper of the last chunk: shrinks the end-of-kernel
    # compute-chain drain that no remaining DMA can hide
    tail_sizes = []
    left = e
    while left > 2 * e_tail:
        tail_sizes.append(e_tail)
        left -= e_tail
    while left > 2 * (e_tail // 4):
        tail_sizes.append(e_tail // 2)
        left -= e_tail // 2
    tail_sizes += [e_tail // 4] * (left // (e_tail // 4))
    assert sum(tail_sizes) == e, (tail_sizes, e)
    tail_aps = []
    row0 = (n_chunks - 1) * P * e
    for sz in tail_sizes:
        zz1 = xy[0, row0:row0 + P * sz, :].rearrange(
            "(p e) d -> p (e d)", p=P, e=sz)
        zz2 = xy[1, row0:row0 + P * sz, :].rearrange(
            "(p e) d -> p (e d)", p=P, e=sz)
        oo = out[row0:row0 + P * sz].rearrange("(p e) -> p e", p=P, e=sz)
        tail_aps.append((zz1, zz2, oo, sz))
        row0 += P * sz

    # --- software-pipelined stages -------------------------------------
    # Each engine executes its instruction stream IN ORDER, so the
    # per-piece chain sub->abs->cmp->ident must be emitted with a lag-1
    # skew across pieces or DVE and ACT ping-pong (each waits for the
    # other's op on the same piece). Emission order per loop step i:
    #   load+sub(i) ; abs(i-1) ; cmp(i-1) ; ident+store(i-2)
    # giving DVE = [S0 S1 C0 S2 C1 ...], ACT = [A0 A1 F0 A2 F1 ...]:
    # every instruction's producer ran at least one slot earlier.

    def stage_load_sub(st):
        io, tp, in1_ap, in2_ap, ecur, tag = (
            st["io"], st["tp"], st["in1"], st["in2"], st["e"], st["tag"])
        t = io.tile([P, 4 * ecur], f32, tag="xy" + tag)
        nc.sync.dma_start(out=t[:, 0:2 * ecur], in_=in1_ap)
        nc.sync.dma_start(out=t[:, 2 * ecur:4 * ecur], in_=in2_ap)
        v = t[:].rearrange("p (s e d) -> p s e d", s=2, d=2)
        d_ = tp.tile([P, 2 * ecur], f32, tag="d" + tag)
        # one DVE subtract over both planes; (dx, dy) stay interleaved
        nc.vector.tensor_tensor(
            out=d_[:], in0=v[:, 0, :, :], in1=v[:, 1, :, :],
            op=Alu.subtract)
        st["d_"] = d_

    def stage_abs(st):
        d_ = st["d_"]
        nc.scalar.activation(out=d_[:], in_=d_[:], func=Act.Abs)

    def stage_cmp(st):
        tp, ecur, tag, d_ = st["tp"], st["e"], st["tag"], st["d_"]
        dv = d_[:].rearrange("p (e d) -> p e d", d=2)
        # cx = (|dx| < 2) * (HI - LO)  ->  {0.98, 0}
        cx = tp.tile([P, ecur], f32, tag="cx" + tag)
        nc.vector.tensor_scalar(
            out=cx[:], in0=dv[:, :, 0], scalar1=X_THRESH, scalar2=HI - LO,
            op0=Alu.is_lt, op1=Alu.mult)
        # res0 = (|dy| <= 0.1) * cx  (one fused DVE op)
        res0 = tp.tile([P, ecur], f32, tag="res0" + tag)
        nc.vector.scalar_tensor_tensor(
            out=res0[:], in0=dv[:, :, 1], scalar=float(np.float32(0.1)),
            in1=cx[:], op0=Alu.is_le, op1=Alu.mult)
        st["res0"] = res0

    def stage_fin(st, lo_ap):
        res0, out_ap = st["res0"], st["out"]
        # res = res0 + LO on ACT (in place): keeps the final select off
        # DVE, and the store is issued by the same engine right after
        nc.scalar.activation(out=res0[:], in_=res0[:], func=Act.Identity,
                             bias=lo_ap)
        # store on the ACT HWDGE queue: doesn't block the input stream
        nc.scalar.dma_start(out=out_ap, in_=res0[:])

    with TileContext(nc) as tc:
        from contextlib import ExitStack
        with ExitStack() as ctx:
            cp = ctx.enter_context(tc.tile_pool(name="const", bufs=1))
            lo_t = cp.tile([P, 1], f32, tag="lo")
            nc.gpsimd.memset(lo_t[:], LO)
            io = ctx.enter_context(tc.tile_pool(name="io", bufs=io_bufs))
            tp = ctx.enter_context(tc.tile_pool(name="tmp", bufs=tmp_bufs))
            tio = (
                ctx.enter_context(tc.tile_pool(name="tio", bufs=tail_bufs))
                if tail_bufs else io
            )
            ttp = (
                ctx.enter_context(tc.tile_pool(name="ttp", bufs=tail_bufs))
                if tail_bufs else tp
            )
            pieces = [
                dict(io=io, tp=tp, in1=z1c[c], in2=z2c[c], out=outt[c],
                     e=e, tag="")
                for c in range(n_chunks - 1)
            ] + [
                dict(io=tio, tp=ttp, in1=zz1, in2=zz2, out=oo, e=sz,
                     tag="t" if tail_bufs else "")
                for zz1, zz2, oo, sz in tail_aps
            ]
            n = len(pieces)
            for i in range(n + 2):
                if i < n:
                    stage_load_sub(pieces[i])
                if 1 <= i <= n:
                    stage_abs(pieces[i - 1])
                    stage_cmp(pieces[i - 1])
                if 2 <= i:
                    stage_fin(pieces[i - 2], lo_t[:])

    nc.finalize()
    return nc


def _pack(z_1: np.ndarray, z_2: np.ndarray) -> np.ndarray:
    """Shard prep: per core, planes [2, N, 2] = (z1 xy pairs, z2 xy pairs)."""
    arr = np.empty((M, 2, N, 2), dtype=np.float32)
    for i in range(M):
        arr[i, 0] = z_1[i * N:(i + 1) * N, 4:6]
        arr[i, 1] = z_2[i * N:(i + 1) * N, 4:6]
    return arr


def _run(z_1: np.ndarray, z_2: np.ndarray, trace: bool = False, **bkw):
    from concourse.bass_utils import run_bass_kernel_spmd

    key = tuple(sorted(bkw.items()))
    if key not in _cache:
        _cache[key] = _build(**bkw)
    nc = _cache[key]

    arr = _pack(np.asarray(z_1, dtype=np.float32),
                np.asarray(z_2, dtype=np.float32))
    in_maps = [{"xy": arr[i]} for i in range(M)]
    r = run_bass_kernel_spmd(nc, in_maps, list(range(M)), trace=trace)
    out = np.concatenate([r.results[i]["out"] for i in range(M)], axis=0)
    return out, r


def kernel(z_1: np.ndarray, z_2: np.ndarray) -> np.ndarray:
    out, _ = _run(z_1, z_2, trace=False)
    return out
